# revision 27
# baseline (speedup 1.0000x reference)
"""Multi-level-KV attention (MKA) kernel for 8 TRN2 NeuronCores.

Math shortcut: memory levels L2 (mean-pooled, all keys identical) and L3
(zeros) have exactly uniform attention, so their contributions collapse to
per-batch constant vectors folded into 3 augmented rows of the output
projection. Only L1 needs real attention.

Sharding: core c -> batch b=c//4, head-quad h=c%4 (4 of 16 heads; column
slice 256h:256h+256 of the q/k/v projections, row slice of Wo). The routing
MLP is column-sharded with a tiny [3,2048] logits AllReduce per batch group.
Each core returns a partial [T,C] output; the host sums each group of 4.

Schedule notes (v2): inputs arrive in a handful of batched DMAs so the PE
starts within a few us; the attention kt-loop is software-pipelined (AV
matmuls consume the previous iteration's exp tiles) so the PE stream has no
intra-loop waits; all per-quad softmax normalization runs off the PE via a
DRAM-bounce transpose + one [128,16] reciprocal; the normalizer broadcast
(wb) and attT writeback (mo) for quad g are emitted mid-quad g+1 so their
DMA chain never stalls the PE. Output partials are bf16, summed on host.
"""
import sys

import numpy as np

_REPO = "/opt/trn_rl_repo"

B, T, C, H = 2, 2048, 1024, 16
D = C // H
P = 128
SCALE = D ** -0.5
NHL = 4  # heads per core


def _setup_env():
    if _REPO not in sys.path:
        sys.path.insert(0, _REPO)
    import concourse.tile as tile
    from concourse import mybir
    from concourse.vector_clock import ScopedClock

    if getattr(tile.TileContext, "_drain_patched", False):
        return

    # This walrus build rejects CTRL instructions (Drain) carrying more than
    # one sync wait; move the end-of-kernel drain's waits onto 1-wait nops.
    def _drain_and_barrier_split(self, tick_clock, wait_clock):
        carrier = self.nc.sync.nop(nofuse=True, hint="drain_wait_carrier")
        wait_clock.add_sem_waits(
            carrier.ins, ScopedClock({None: tick_clock.global_clock})
        )
        si = carrier.ins.sync_info
        waits = list(si.on_wait) if si is not None and si.on_wait else []
        if si is not None:
            si.on_wait = waits[:1]
        for w in waits[1:]:
            nop = self.nc.sync.nop(nofuse=True, hint="drain_wait_carrier")
            nop.ins.sync_info = mybir.SyncInfo(on_wait=[w], on_update=[])
        self.nc.sync.drain()
        self.nc.all_engine_barrier()
        assert self.sems is not None
        popped = self.nc._tile_sem_poison_stack.pop()
        assert popped is self._sem_poison
        self.nc.clear_and_free_semaphores(list(self.sems.allocated().values()))
        self.nc.all_engine_barrier()

    tile.TileContext._drain_and_barrier = _drain_and_barrier_split
    tile.TileContext._drain_patched = True


def build_nc():
    _setup_env()
    from contextlib import ExitStack

    import concourse.bass as bass
    import concourse.tile as tile
    from concourse import mybir

    f32 = mybir.dt.float32
    bf = mybir.dt.bfloat16
    Exp = mybir.ActivationFunctionType.Exp
    Tanh = mybir.ActivationFunctionType.Tanh
    Copy = mybir.ActivationFunctionType.Copy

    nc = bass.Bass()
    xT = nc.dram_tensor("xT", [C, T], bf, kind="ExternalInput")
    wq = nc.dram_tensor("wq", [C, 256], bf, kind="ExternalInput")
    wk = nc.dram_tensor("wk", [C, 256], bf, kind="ExternalInput")
    wv = nc.dram_tensor("wv", [C, 256], bf, kind="ExternalInput")
    wr1 = nc.dram_tensor("wr1", [C, 256], bf, kind="ExternalInput")
    wr2 = nc.dram_tensor("wr2", [256, 3], bf, kind="ExternalInput")
    wo = nc.dram_tensor("wo", [256, C], bf, kind="ExternalInput")
    crows = nc.dram_tensor("crows", [3, C], bf, kind="ExternalInput")
    bq2 = nc.dram_tensor("bq2", [2, P, 1], f32, kind="ExternalInput")
    br1t = nc.dram_tensor("br1t", [2, P, 1], f32, kind="ExternalInput")
    br2z = nc.dram_tensor("br2z", [3, 1], f32, kind="ExternalInput")
    outd = nc.dram_tensor("out", [T, C], bf, kind="ExternalOutput")

    with tile.TileContext(nc) as tc, ExitStack() as ctx:
        pp = ctx.enter_context(tc.tile_pool(name="persist", bufs=1))
        exp_pool = ctx.enter_context(tc.tile_pool(name="expool", bufs=6))
        scp = ctx.enter_context(tc.tile_pool(name="scp", bufs=1, space="PSUM"))
        upp = ctx.enter_context(tc.tile_pool(name="upp", bufs=4, space="PSUM"))
        stg = ctx.enter_context(tc.tile_pool(name="stg", bufs=5))
        usbp = ctx.enter_context(tc.tile_pool(name="usbp", bufs=9))
        oup = ctx.enter_context(tc.tile_pool(name="oup", bufs=3))
        drp = ctx.enter_context(tc.tile_pool(name="drm", bufs=1, space="DRAM"))

        # ---- batched persistent loads -------------------------------------
        # xt4[ch][p, k, t] = xT[k*128+p, ch*512+t]  (token-chunk major)
        # small tensors first: the tanh bias must not queue behind megabyte
        # loads in the Sync FIFO (it gated the whole routing chain once)
        bq_sb = pp.tile([P, 2], f32, name="bq_sb")
        nc.sync.dma_start(out=bq_sb, in_=bq2.rearrange("a p o -> p (a o)"))
        br1_sb = pp.tile([P, 2], f32, name="br1_sb")
        nc.sync.dma_start(out=br1_sb, in_=br1t.rearrange("a p o -> p (a o)"))
        br2_sb = pp.tile([3, 1], f32, name="br2_sb")
        nc.sync.dma_start(out=br2_sb, in_=br2z[:, :])
        wr2_sb = pp.tile([P, 2, 3], bf, name="wr2_sb")
        nc.sync.dma_start(out=wr2_sb, in_=wr2.rearrange("(a p) b -> p a b", p=P))
        crow_sb = pp.tile([3, C], bf, name="crow_sb")
        nc.sync.dma_start(out=crow_sb, in_=crows[:, :])

        xr = xT.rearrange("(k p) t -> p k t", p=P)
        xt4 = []
        for ch in range(4):
            t_ = pp.tile([P, 8, 512], bf, name=f"xt4_{ch}")
            xt4.append(t_)
        nc.sync.dma_start(out=xt4[0], in_=xr[:, :, 0:512])
        wr1_sb = pp.tile([P, 8, 256], bf, name="wr1sb")
        nc.sync.dma_start(out=wr1_sb, in_=wr1.rearrange("(k p) c -> p k c", p=P))
        for ch in range(1, 4):
            nc.sync.dma_start(
                out=xt4[ch], in_=xr[:, :, ch * 512:(ch + 1) * 512]
            )
        wq_sb = pp.tile([P, 8, 256], bf, name="wqsb")
        nc.sync.dma_start(out=wq_sb, in_=wq.rearrange("(k p) c -> p k c", p=P))
        wk_sb = pp.tile([P, 8, 256], bf, name="wksb")
        nc.sync.dma_start(out=wk_sb, in_=wk.rearrange("(k p) c -> p k c", p=P))
        wv_sb = pp.tile([P, 8, 256], bf, name="wvsb")
        nc.sync.dma_start(out=wv_sb, in_=wv.rearrange("(k p) c -> p k c", p=P))
        wo_sb = pp.tile([P, 2, C], bf, name="wosb")
        nc.sync.dma_start(out=wo_sb, in_=wo.rearrange("(j p) c -> p j c", p=P))

        qT2 = [pp.tile([P, T], bf, name=f"qT{p}") for p in range(2)]
        kT2 = [pp.tile([P, T], bf, name=f"kT{p}") for p in range(2)]
        attT = [pp.tile([P, T], bf, name=f"attT{p}") for p in range(2)]
        vt = [pp.tile([P, NHL, 65], bf, name=f"vt{j}") for j in range(16)]
        gt = [pp.tile([P, T], bf, name=f"gt{m}") for m in range(2)]
        e_f = pp.tile([3, T], f32, name="e_f")
        e_b = pp.tile([3, T], bf, name="e_b")
        lg_sb = pp.tile([3, T], f32, name="lg_sb")
        lgr_sb = pp.tile([3, T], f32, name="lgr_sb")
        e3t = pp.tile([P, 48], f32, name="e3t")
        s3rt = pp.tile([P, 16], f32, name="s3rt")
        e1q = [pp.tile([16, P], f32, name=f"e1q{h}") for h in range(2)]
        ones1 = pp.tile([1, 64], bf, name="ones1")
        nc.vector.memset(ones1, 1.0)

        # ---- routing: gT = tanh(Wr1_blk^T xT), partial logits, AllReduce --
        for m in range(2):
            for ch in range(4):
                g_ps = upp.tile([P, 512], f32, tag="u", name="g_ps")
                for kt in range(8):
                    nc.tensor.matmul(
                        g_ps,
                        wr1_sb[:, kt, m * P:(m + 1) * P],
                        xt4[ch][:, kt, :],
                        start=(kt == 0),
                        stop=(kt == 7),
                    )
                nc.scalar.activation(
                    out=gt[m][:, ch * 512:(ch + 1) * 512],
                    in_=g_ps,
                    func=Tanh,
                    bias=br1_sb[:, m:m + 1],
                    scale=1.0,
                )
        # ---- projections: qT/kT (heads stacked in pairs), v_aug.
        # The routing logits + AllReduce are emitted after two q/k chunk
        # pairs so the PE queue never head-blocks on the gt tanh latency. ---
        def emit_qk(p_, ch):
            q_ps = upp.tile([P, 512], f32, tag="u", name="q_ps")
            for kt in range(8):
                nc.tensor.matmul(
                    q_ps,
                    wq_sb[:, kt, p_ * P:(p_ + 1) * P],
                    xt4[ch][:, kt, :],
                    start=(kt == 0),
                    stop=(kt == 7),
                )
            nc.vector.tensor_scalar_add(
                out=qT2[p_][:, ch * 512:(ch + 1) * 512],
                in0=q_ps,
                scalar1=bq_sb[:, p_:p_ + 1],
            )
            k_ps = upp.tile([P, 512], f32, tag="u", name="k_ps")
            for kt in range(8):
                nc.tensor.matmul(
                    k_ps,
                    wk_sb[:, kt, p_ * P:(p_ + 1) * P],
                    xt4[ch][:, kt, :],
                    start=(kt == 0),
                    stop=(kt == 7),
                )
            nc.vector.tensor_copy(
                out=kT2[p_][:, ch * 512:(ch + 1) * 512], in_=k_ps
            )

        emit_qk(0, 0)
        for ch in range(4):
            lg_ps = upp.tile([3, 512], f32, tag="u", name="lg_ps")
            for a in range(2):
                nc.tensor.matmul(
                    lg_ps,
                    wr2_sb[:, a, :],
                    gt[a][:, ch * 512:(ch + 1) * 512],
                    start=(a == 0),
                    stop=(a == 1),
                )
            nc.vector.tensor_scalar_add(
                out=lg_sb[:, ch * 512:(ch + 1) * 512],
                in0=lg_ps,
                scalar1=br2_sb,
            )
        lg_in = drp.tile([3, T], f32, name="lg_in")
        lg_out = drp.tile([3, T], f32, name="lg_out")
        nc.sync.dma_start(out=lg_in, in_=lg_sb)
        nc.gpsimd.collective_compute(
            "AllReduce",
            mybir.AluOpType.add,
            replica_groups=[[0, 1, 2, 3], [4, 5, 6, 7]],
            ins=[lg_in.opt()],
            outs=[lg_out.opt()],
        )
        nc.sync.dma_start(out=lgr_sb, in_=lg_out)
        for ch in range(1, 4):
            emit_qk(0, ch)
        for ch in range(4):
            emit_qk(1, ch)
        for j in range(16):
            v_ps = upp.tile([P, 256], f32, tag="u", name="v_ps")
            for kt in range(8):
                nc.tensor.matmul(
                    v_ps,
                    xt4[j // 4][:, kt, (j % 4) * P:(j % 4) * P + P],
                    wv_sb[:, kt, :],
                    start=(kt == 0),
                    stop=(kt == 7),
                )
            nc.vector.tensor_copy(
                out=vt[j][:, :, 0:64],
                in_=v_ps.rearrange("p (h d) -> p h d", h=NHL),
            )
            nc.vector.memset(vt[j][:, :, 64:65], 1.0)

        # routing epilogue pieces — emitted staged inside quad g1 so the
        # AllReduce latency never head-blocks the Scalar/Vector FIFOs.
        e3d = drp.tile([3, T], f32, name="e3d")

        def emit_epi_a():
            nc.scalar.activation(out=e_f, in_=lgr_sb, func=Exp, scale=1.0)
            nc.vector.tensor_copy(out=e_b, in_=e_f)
            nc.sync.dma_start(out=e3d, in_=e_f)
            nc.sync.dma_start(
                out=e3t, in_=e3d.rearrange("a (i p) -> p (a i)", p=P)
            )

        def emit_epi_b():
            nc.vector.tensor_add(out=s3rt, in0=e3t[:, 0:16], in1=e3t[:, 16:32])
            nc.vector.tensor_add(out=s3rt, in0=s3rt, in1=e3t[:, 32:48])
            nc.vector.reciprocal(out=s3rt, in_=s3rt)
            # e1 in quad-drain form: partition (h2,qc,i), free p (see drain)
            for h in range(2):
                src = e3d[0:1, h * 1024:(h + 1) * 1024]
                for rep in range(2):
                    nc.sync.dma_start(
                        out=e1q[h][rep * 8:rep * 8 + 8, :],
                        in_=src.rearrange(
                            "a (q i p) -> (a q i) p", q=2, i=4, p=P
                        ),
                    )

        # ---- attention: half-major quads, software-pipelined kt loop ------
        quads = [(h, p_) for h in range(2) for p_ in range(2)]
        fin = {}   # quad idx -> (srt, half) awaiting the e1-mul + return DMAs
        pend = {}  # quad idx -> (usb list, w1row, half, p_) awaiting wb+mo

        def emit_w1fin(gi):
            srt, half = fin.pop(gi)
            w1s = stg.tile([16, P], bf, tag="w1s", name="w1s")
            nc.vector.tensor_mul(out=w1s, in0=srt, in1=e1q[half])
            wqd = drp.tile([2048], bf, name=f"wqd{gi}")
            nc.sync.dma_start(
                out=wqd.rearrange("(r p) -> r p", r=16), in_=w1s
            )
            w1row = stg.tile([1, 2048], bf, tag="w1row", name="w1row")
            nc.sync.dma_start(
                out=w1row, in_=wqd.rearrange("(a x) -> a x", a=1)
            )
            usb, half_, p_ = pend[gi]
            pend[gi] = (usb, w1row, half_, p_)

        def emit_av(p_, u_ps, exs, kt1):
            for h2 in range(2):
                for qc in range(2):
                    nc.tensor.matmul(
                        u_ps[h2 * 2 + qc],
                        vt[kt1][:, p_ * 2 + h2, :],
                        exs[h2][:, qc * 512:(qc + 1) * 512],
                        start=(kt1 == 0),
                        stop=(kt1 == 15),
                    )

        def emit_wbmo(gi):
            usb, w1row, half, p_ = pend.pop(gi)  # w1fin(gi) ran before this
            for h2 in range(2):
                for qc in range(2):
                    c = h2 * 2 + qc
                    c0 = half * 1024 + qc * 512
                    wb = scp.tile([64, 512], f32, tag="sc0", name="wb")
                    nc.tensor.matmul(
                        wb, ones1, w1row[0:1, c * 512:(c + 1) * 512],
                        start=True, stop=True,
                    )
                    nc.vector.tensor_mul(
                        out=attT[p_][h2 * 64:(h2 + 1) * 64, c0:c0 + 512],
                        in0=usb[c][0:64, :],
                        in1=wb,
                    )

        # Schraudolph fast-exp constants: exp(s*SCALE) ~= bitcast_f32(
        # int32(A*s + B)); B centers the relative error (rms ~1.8%), which
        # largely cancels in the softmax ratio. Applied to half of one head's
        # scores per iteration to split exp work between ScalarE and VectorE.
        S_A = SCALE * 12102203.1615614666
        S_B = 1064866805.0
        Mult = mybir.AluOpType.mult
        Add = mybir.AluOpType.add
        DCUT = 384  # sc1 cols [0:DCUT] exp'd on ScalarE, rest on VectorE

        for gi, (half, p_) in enumerate(quads):
            u_ps = [
                upp.tile([65, 512], f32, tag="u", name="u_ps")
                for _ in range(4)
            ]
            prev = None
            for kt in range(16):
                sc0 = scp.tile([P, 1024], f32, tag="sc0", name="sc0")
                sc1 = scp.tile([P, 1024], f32, tag="sc1", name="sc1")
                for n2 in range(2):
                    c0 = half * 1024 + n2 * 512
                    nc.tensor.matmul(
                        sc0[:, n2 * 512:(n2 + 1) * 512],
                        kT2[p_][0:64, kt * P:(kt + 1) * P],
                        qT2[p_][0:64, c0:c0 + 512],
                        start=True,
                        stop=True,
                    )
                    nc.tensor.matmul(
                        sc1[:, n2 * 512:(n2 + 1) * 512],
                        kT2[p_][64:128, kt * P:(kt + 1) * P],
                        qT2[p_][64:128, c0:c0 + 512],
                        start=True,
                        stop=True,
                    )
                if prev is not None:
                    emit_av(p_, u_ps, prev, kt - 1)
                ex0 = exp_pool.tile([P, 1024], bf, tag="ex", name="ext")
                nc.scalar.activation(out=ex0, in_=sc0, func=Exp, scale=SCALE)
                ex1 = exp_pool.tile([P, 1024], bf, tag="ex", name="ext")
                nc.scalar.activation(
                    out=ex1[:, 0:DCUT], in_=sc1[:, 0:DCUT], func=Exp, scale=SCALE
                )
                t32 = stg.tile([P, 1024 - DCUT], mybir.dt.int32, tag="t32",
                               name="t32")
                nc.vector.tensor_scalar(
                    out=t32, in0=sc1[:, DCUT:1024],
                    scalar1=S_A, scalar2=S_B, op0=Mult, op1=Add,
                )
                nc.vector.tensor_copy(
                    out=ex1[:, DCUT:1024], in_=t32[:, :].bitcast(f32)
                )
                prev = (ex0, ex1)
                if gi == 1:
                    if kt == 4:
                        emit_epi_a()
                    elif kt == 8:
                        emit_epi_b()
                    elif kt == 10:
                        emit_w1fin(0)
                if kt == 2 and (gi - 2) in pend:
                    emit_wbmo(gi - 2)
                if kt == 12 and gi >= 2 and (gi - 1) in pend:
                    emit_wbmo(gi - 1)
            emit_av(p_, u_ps, prev, 15)

            # quad drain: copy u out of PSUM; build w1 = e1/S off the PE.
            # All DMA hops use per-partition-contiguous runs (>=256B): the
            # sums go to DRAM as 4 rows, come back as [16,128] (partition =
            # (h2,qc,i), free = p), reciprocal+e1-mul run 16 lanes wide, and
            # the result stores straight back in (h2,qc,i,p)-linear order.
            usb = []
            sq = drp.tile([4, 512], bf, name=f"sq{gi}")
            for c in range(4):
                u_sb = usbp.tile([65, 512], bf, tag="usb", name="usb")
                nc.vector.tensor_copy(out=u_sb, in_=u_ps[c])
                nc.sync.dma_start(out=sq[c:c + 1, :], in_=u_sb[64:65, :])
                usb.append(u_sb)
            srt_b = stg.tile([16, P], bf, tag="srtb", name="srtb")
            nc.sync.dma_start(
                out=srt_b, in_=sq.rearrange("c (i p) -> (c i) p", p=P)
            )
            srt = stg.tile([16, P], f32, tag="srt", name="srt")
            nc.vector.reciprocal(out=srt, in_=srt_b)
            fin[gi] = (srt, half)
            pend[gi] = (usb, half, p_)
            if gi >= 1:
                emit_w1fin(gi)

        # ---- output projection (augmented with routing const rows) -------
        def emit_po(qt):
            po = scp.tile(
                [P, 1024], f32, tag=("sc0" if qt % 2 == 0 else "sc1"), name="po"
            )
            for n2 in range(2):
                ns = slice(n2 * 512, (n2 + 1) * 512)
                nc.tensor.matmul(
                    po[:, ns],
                    attT[0][:, qt * P:(qt + 1) * P],
                    wo_sb[:, 0, ns],
                    start=True,
                    stop=False,
                )
                nc.tensor.matmul(
                    po[:, ns],
                    attT[1][:, qt * P:(qt + 1) * P],
                    wo_sb[:, 1, ns],
                    start=False,
                    stop=False,
                )
                nc.tensor.matmul(
                    po[:, ns],
                    e_b[:, qt * P:(qt + 1) * P],
                    crow_sb[:, ns],
                    start=False,
                    stop=True,
                )
            ou = oup.tile([P, C], bf, tag="ou", name="ou")
            nc.scalar.activation(
                out=ou, in_=po, func=Copy, scale=s3rt[:, qt:qt + 1], bias=0.0
            )
            nc.sync.dma_start(out=outd[qt * P:(qt + 1) * P, :], in_=ou)

        for qt in range(8):
            emit_po(qt)
        emit_wbmo(3)
        for qt in range(8, 16):
            emit_po(qt)

    _split_excess_waits(nc, mybir)
    return nc


def _split_excess_waits(nc, mybir, keep=1):
    """Walrus in this build accepts at most 1 sync wait per instruction;
    move excess waits onto single-wait nops inserted just before, on the
    same engine (same-engine program order makes this equivalent)."""
    n_extra = 0
    for bb in nc.main_func.blocks:
        out = []
        for inst in bb.instructions:
            si = inst.sync_info
            if si is not None and si.on_wait and len(si.on_wait) > keep:
                waits = list(si.on_wait)
                si.on_wait = waits[:keep]
                for w in waits[keep:]:
                    n_extra += 1
                    nop = mybir.InstNoOp(
                        name=f"wsplit_{n_extra}_{inst.name}",
                        engine=inst.engine,
                        ins=[],
                        outs=[],
                        sync_info=mybir.SyncInfo(on_wait=[w], on_update=[]),
                    )
                    out.append(nop)
            out.append(inst)
        bb.instructions[:] = out
    return nc


def make_in_maps(inputs):
    import ml_dtypes

    bf16 = ml_dtypes.bfloat16
    f32 = np.float32
    x = np.asarray(inputs["hidden_states"], f32)
    Wq = np.asarray(inputs["Wq"], f32)
    bq = np.asarray(inputs["bq"], f32)
    Wk = np.asarray(inputs["Wk"], f32)
    Wv = np.asarray(inputs["Wv"], f32)
    bv = np.asarray(inputs["bv"], f32)
    Wo = np.asarray(inputs["Wo"], f32)
    bo = np.asarray(inputs["bo"], f32)
    Wr1 = np.asarray(inputs["Wr1"], f32)
    br1 = np.asarray(inputs["br1"], f32)
    Wr2 = np.asarray(inputs["Wr2"], f32)
    br2 = np.asarray(inputs["br2"], f32)

    xTb = [np.ascontiguousarray(x[b].T).astype(bf16) for b in range(B)]
    c3 = bv @ Wo + bo
    c2 = []
    for b in range(B):
        mean = x[b].mean(axis=0)
        c2.append((mean @ Wv + bv) @ Wo + bo)

    def cut(w, cs, ce):
        return np.ascontiguousarray(w[:, cs:ce]).astype(bf16)

    in_maps = []
    for c in range(8):
        b, h = divmod(c, 4)
        cs, ce = h * 256, h * 256 + 256
        crows_np = np.zeros((3, C), f32)
        crows_np[0] = bv[cs:ce] @ Wo[cs:ce, :] + (bo if h == 0 else 0.0)
        if h == 0:
            crows_np[1] = c2[b]
            crows_np[2] = c3
        in_maps.append({
            "xT": xTb[b],
            "wq": cut(Wq, cs, ce),
            "wk": cut(Wk, cs, ce),
            "wv": cut(Wv, cs, ce),
            "wr1": cut(Wr1, cs, ce),
            "wr2": np.ascontiguousarray(Wr2[cs:ce, :]).astype(bf16),
            "wo": np.ascontiguousarray(Wo[cs:ce, :]).astype(bf16),
            "crows": crows_np.astype(bf16),
            "bq2": np.ascontiguousarray(bq[cs:ce].reshape(2, P, 1)),
            "br1t": np.ascontiguousarray(br1[cs:ce].reshape(2, P, 1)),
            "br2z": np.ascontiguousarray(
                (br2 if h == 0 else np.zeros(3, f32)).reshape(3, 1)
            ),
        })
    return in_maps


_NC = None


def kernel(**inputs):
    global _NC
    _setup_env()
    from concourse.bass_utils import run_bass_kernel_spmd

    if _NC is None:
        _NC = build_nc()
    in_maps = make_in_maps(inputs)
    res = run_bass_kernel_spmd(_NC, in_maps, core_ids=list(range(8)))
    outs = [res.results[c]["out"].astype(np.float32) for c in range(8)]
    full = np.stack(
        [outs[0] + outs[1] + outs[2] + outs[3],
         outs[4] + outs[5] + outs[6] + outs[7]],
        axis=0,
    )
    return full


# revision 31
# speedup vs baseline: 3.9591x; 3.9591x over previous
"""Multi-level-KV attention (MKA) kernel for 8 TRN2 NeuronCores.

Math shortcut: memory levels L2 (mean-pooled, all keys identical) and L3
(zeros) have exactly uniform attention, so their contributions collapse to
per-batch constant vectors folded into 3 augmented rows of the output
projection. Only L1 needs real attention.

Sharding: core c -> batch b=c//4, head-quad h=c%4 (4 of 16 heads; column
slice 256h:256h+256 of the q/k/v projections, row slice of Wo). The routing
MLP is column-sharded with a tiny [3,2048] logits AllReduce per batch group.
Each core returns a partial [T,C] output; the host sums each group of 4.

Schedule notes (v2): inputs arrive in a handful of batched DMAs so the PE
starts within a few us; the attention kt-loop is software-pipelined (AV
matmuls consume the previous iteration's exp tiles) so the PE stream has no
intra-loop waits; all per-quad softmax normalization runs off the PE via a
DRAM-bounce transpose + one [128,16] reciprocal; the normalizer broadcast
(wb) and attT writeback (mo) for quad g are emitted mid-quad g+1 so their
DMA chain never stalls the PE. Output partials are bf16, summed on host.
"""
import sys

import numpy as np

_REPO = "/opt/trn_rl_repo"

B, T, C, H = 2, 2048, 1024, 16
D = C // H
P = 128
SCALE = D ** -0.5
NHL = 4  # heads per core


def _setup_env():
    if _REPO not in sys.path:
        sys.path.insert(0, _REPO)
    import concourse.tile as tile
    from concourse import mybir
    from concourse.vector_clock import ScopedClock

    if getattr(tile.TileContext, "_drain_patched", False):
        return

    # This walrus build rejects CTRL instructions (Drain) carrying more than
    # one sync wait; move the end-of-kernel drain's waits onto 1-wait nops.
    def _drain_and_barrier_split(self, tick_clock, wait_clock):
        carrier = self.nc.sync.nop(nofuse=True, hint="drain_wait_carrier")
        wait_clock.add_sem_waits(
            carrier.ins, ScopedClock({None: tick_clock.global_clock})
        )
        si = carrier.ins.sync_info
        waits = list(si.on_wait) if si is not None and si.on_wait else []
        if si is not None:
            si.on_wait = waits[:1]
        for w in waits[1:]:
            nop = self.nc.sync.nop(nofuse=True, hint="drain_wait_carrier")
            nop.ins.sync_info = mybir.SyncInfo(on_wait=[w], on_update=[])
        self.nc.sync.drain()
        self.nc.all_engine_barrier()
        assert self.sems is not None
        popped = self.nc._tile_sem_poison_stack.pop()
        assert popped is self._sem_poison
        self.nc.clear_and_free_semaphores(list(self.sems.allocated().values()))
        self.nc.all_engine_barrier()

    tile.TileContext._drain_and_barrier = _drain_and_barrier_split
    tile.TileContext._drain_patched = True


def build_nc():
    _setup_env()
    from contextlib import ExitStack

    import concourse.bass as bass
    import concourse.tile as tile
    from concourse import mybir

    f32 = mybir.dt.float32
    bf = mybir.dt.bfloat16
    Exp = mybir.ActivationFunctionType.Exp
    Tanh = mybir.ActivationFunctionType.Tanh
    Copy = mybir.ActivationFunctionType.Copy

    nc = bass.Bass()
    xT = nc.dram_tensor("xT", [C, T], bf, kind="ExternalInput")
    wq = nc.dram_tensor("wq", [C, 256], bf, kind="ExternalInput")
    wk = nc.dram_tensor("wk", [C, 256], bf, kind="ExternalInput")
    wv = nc.dram_tensor("wv", [C, 256], bf, kind="ExternalInput")
    wr1 = nc.dram_tensor("wr1", [C, 256], bf, kind="ExternalInput")
    wr2 = nc.dram_tensor("wr2", [256, 3], bf, kind="ExternalInput")
    wo = nc.dram_tensor("wo", [256, C], bf, kind="ExternalInput")
    crows = nc.dram_tensor("crows", [3, C], bf, kind="ExternalInput")
    bq2 = nc.dram_tensor("bq2", [2, P, 1], f32, kind="ExternalInput")
    br1t = nc.dram_tensor("br1t", [2, P, 1], f32, kind="ExternalInput")
    br2z = nc.dram_tensor("br2z", [3, 1], f32, kind="ExternalInput")
    outd = nc.dram_tensor("out", [T, C], bf, kind="ExternalOutput")

    with tile.TileContext(nc) as tc, ExitStack() as ctx:
        pp = ctx.enter_context(tc.tile_pool(name="persist", bufs=1))
        exp_pool = ctx.enter_context(tc.tile_pool(name="expool", bufs=6))
        scp = ctx.enter_context(tc.tile_pool(name="scp", bufs=1, space="PSUM"))
        upp = ctx.enter_context(tc.tile_pool(name="upp", bufs=4, space="PSUM"))
        stg = ctx.enter_context(tc.tile_pool(name="stg", bufs=5))
        usbp = ctx.enter_context(tc.tile_pool(name="usbp", bufs=9))
        oup = ctx.enter_context(tc.tile_pool(name="oup", bufs=3))
        drp = ctx.enter_context(tc.tile_pool(name="drm", bufs=1, space="DRAM"))

        # ---- batched persistent loads -------------------------------------
        # xt4[ch][p, k, t] = xT[k*128+p, ch*512+t]  (token-chunk major)
        # small tensors first: the tanh bias must not queue behind megabyte
        # loads in the Sync FIFO (it gated the whole routing chain once)
        bq_sb = pp.tile([P, 2], f32, name="bq_sb")
        nc.sync.dma_start(out=bq_sb, in_=bq2.rearrange("a p o -> p (a o)"))
        br1_sb = pp.tile([P, 2], f32, name="br1_sb")
        nc.sync.dma_start(out=br1_sb, in_=br1t.rearrange("a p o -> p (a o)"))
        br2_sb = pp.tile([3, 1], f32, name="br2_sb")
        nc.sync.dma_start(out=br2_sb, in_=br2z[:, :])
        wr2_sb = pp.tile([P, 2, 3], bf, name="wr2_sb")
        nc.sync.dma_start(out=wr2_sb, in_=wr2.rearrange("(a p) b -> p a b", p=P))
        crow_sb = pp.tile([3, C], bf, name="crow_sb")
        nc.sync.dma_start(out=crow_sb, in_=crows[:, :])

        xr = xT.rearrange("(k p) t -> p k t", p=P)
        xt4 = []
        for ch in range(4):
            t_ = pp.tile([P, 8, 512], bf, name=f"xt4_{ch}")
            xt4.append(t_)
        nc.sync.dma_start(out=xt4[0], in_=xr[:, :, 0:512])
        wr1_sb = pp.tile([P, 8, 256], bf, name="wr1sb")
        nc.sync.dma_start(out=wr1_sb, in_=wr1.rearrange("(k p) c -> p k c", p=P))
        for ch in range(1, 4):
            nc.sync.dma_start(
                out=xt4[ch], in_=xr[:, :, ch * 512:(ch + 1) * 512]
            )
        wq_sb = pp.tile([P, 8, 256], bf, name="wqsb")
        nc.sync.dma_start(out=wq_sb, in_=wq.rearrange("(k p) c -> p k c", p=P))
        wk_sb = pp.tile([P, 8, 256], bf, name="wksb")
        nc.sync.dma_start(out=wk_sb, in_=wk.rearrange("(k p) c -> p k c", p=P))
        wv_sb = pp.tile([P, 8, 256], bf, name="wvsb")
        nc.sync.dma_start(out=wv_sb, in_=wv.rearrange("(k p) c -> p k c", p=P))
        wo_sb = pp.tile([P, 2, C], bf, name="wosb")
        nc.sync.dma_start(out=wo_sb, in_=wo.rearrange("(j p) c -> p j c", p=P))

        f8 = mybir.dt.float8e4
        qT2 = [pp.tile([P, T], bf, name=f"qT{p}") for p in range(2)]
        kT2 = [pp.tile([P, T], bf, name=f"kT{p}") for p in range(2)]
        attT = [pp.tile([P, T], bf, name=f"attT{p}") for p in range(2)]
        # v in fp8, key-blocks paired for DoubleRow AV; m padded to 68 so the
        # pair stride (4*68 bytes) is 16-aligned as checkMatmultPerfMode wants
        vt8 = [pp.tile([P, 2, NHL, 68], f8, name=f"vt8_{j}") for j in range(8)]
        gt = [pp.tile([P, T], bf, name=f"gt{m}") for m in range(2)]
        e_f = pp.tile([3, T], f32, name="e_f")
        e_b = pp.tile([3, T], bf, name="e_b")
        lg_sb = pp.tile([3, T], f32, name="lg_sb")
        lgr_sb = pp.tile([3, T], f32, name="lgr_sb")
        e3t = pp.tile([P, 48], f32, name="e3t")
        s3rt = pp.tile([P, 16], f32, name="s3rt")
        e1q = [pp.tile([16, P], f32, name=f"e1q{h}") for h in range(2)]
        ones1 = pp.tile([1, 64], bf, name="ones1")
        nc.vector.memset(ones1, 1.0)

        # ---- routing: gT = tanh(Wr1_blk^T xT), partial logits, AllReduce --
        for m in range(2):
            for ch in range(4):
                g_ps = upp.tile([P, 512], f32, tag="u", name="g_ps")
                for kt in range(8):
                    nc.tensor.matmul(
                        g_ps,
                        wr1_sb[:, kt, m * P:(m + 1) * P],
                        xt4[ch][:, kt, :],
                        start=(kt == 0),
                        stop=(kt == 7),
                    )
                nc.scalar.activation(
                    out=gt[m][:, ch * 512:(ch + 1) * 512],
                    in_=g_ps,
                    func=Tanh,
                    bias=br1_sb[:, m:m + 1],
                    scale=1.0,
                )
        # ---- projections: qT/kT (heads stacked in pairs), v_aug.
        # The routing logits + AllReduce are emitted after two q/k chunk
        # pairs so the PE queue never head-blocks on the gt tanh latency. ---
        def emit_qk(p_, ch):
            q_ps = upp.tile([P, 512], f32, tag="u", name="q_ps")
            for kt in range(8):
                nc.tensor.matmul(
                    q_ps,
                    wq_sb[:, kt, p_ * P:(p_ + 1) * P],
                    xt4[ch][:, kt, :],
                    start=(kt == 0),
                    stop=(kt == 7),
                )
            nc.vector.tensor_scalar_add(
                out=qT2[p_][:, ch * 512:(ch + 1) * 512],
                in0=q_ps,
                scalar1=bq_sb[:, p_:p_ + 1],
            )
            k_ps = upp.tile([P, 512], f32, tag="u", name="k_ps")
            for kt in range(8):
                nc.tensor.matmul(
                    k_ps,
                    wk_sb[:, kt, p_ * P:(p_ + 1) * P],
                    xt4[ch][:, kt, :],
                    start=(kt == 0),
                    stop=(kt == 7),
                )
            nc.vector.tensor_copy(
                out=kT2[p_][:, ch * 512:(ch + 1) * 512], in_=k_ps
            )

        emit_qk(0, 0)
        for ch in range(4):
            lg_ps = upp.tile([3, 512], f32, tag="u", name="lg_ps")
            for a in range(2):
                nc.tensor.matmul(
                    lg_ps,
                    wr2_sb[:, a, :],
                    gt[a][:, ch * 512:(ch + 1) * 512],
                    start=(a == 0),
                    stop=(a == 1),
                )
            nc.vector.tensor_scalar_add(
                out=lg_sb[:, ch * 512:(ch + 1) * 512],
                in0=lg_ps,
                scalar1=br2_sb,
            )
        lg_in = drp.tile([3, T], f32, name="lg_in")
        lg_out = drp.tile([3, T], f32, name="lg_out")
        nc.sync.dma_start(out=lg_in, in_=lg_sb)
        nc.gpsimd.collective_compute(
            "AllReduce",
            mybir.AluOpType.add,
            replica_groups=[[0, 1, 2, 3], [4, 5, 6, 7]],
            ins=[lg_in.opt()],
            outs=[lg_out.opt()],
        )
        nc.sync.dma_start(out=lgr_sb, in_=lg_out)
        for ch in range(1, 4):
            emit_qk(0, ch)
        for ch in range(4):
            emit_qk(1, ch)
        for j in range(16):
            v_ps = upp.tile([P, 256], f32, tag="u", name="v_ps")
            for kt in range(8):
                nc.tensor.matmul(
                    v_ps,
                    xt4[j // 4][:, kt, (j % 4) * P:(j % 4) * P + P],
                    wv_sb[:, kt, :],
                    start=(kt == 0),
                    stop=(kt == 7),
                )
            nc.vector.tensor_copy(
                out=vt8[j // 2][:, j % 2, :, 0:64],
                in_=v_ps.rearrange("p (h d) -> p h d", h=NHL),
            )
            nc.vector.memset(vt8[j // 2][:, j % 2, :, 64:65], 1.0)

        # routing epilogue pieces — emitted staged inside quad g1 so the
        # AllReduce latency never head-blocks the Scalar/Vector FIFOs.
        e3d = drp.tile([3, T], f32, name="e3d")

        def emit_epi_a():
            nc.scalar.activation(out=e_f, in_=lgr_sb, func=Exp, scale=1.0)
            nc.vector.tensor_copy(out=e_b, in_=e_f)
            nc.sync.dma_start(out=e3d, in_=e_f)
            nc.sync.dma_start(
                out=e3t, in_=e3d.rearrange("a (i p) -> p (a i)", p=P)
            )

        def emit_epi_b():
            nc.vector.tensor_add(out=s3rt, in0=e3t[:, 0:16], in1=e3t[:, 16:32])
            nc.vector.tensor_add(out=s3rt, in0=s3rt, in1=e3t[:, 32:48])
            nc.vector.reciprocal(out=s3rt, in_=s3rt)
            # e1 in quad-drain form: partition (h2,qc,i), free p (see drain)
            for h in range(2):
                src = e3d[0:1, h * 1024:(h + 1) * 1024]
                for rep in range(2):
                    nc.sync.dma_start(
                        out=e1q[h][rep * 8:rep * 8 + 8, :],
                        in_=src.rearrange(
                            "a (q i p) -> (a q i) p", q=2, i=4, p=P
                        ),
                    )

        # ---- attention: half-major quads, software-pipelined kt loop ------
        quads = [(h, p_) for h in range(2) for p_ in range(2)]
        fin = {}   # quad idx -> (srt, half) awaiting the e1-mul + return DMAs
        pend = {}  # quad idx -> (usb list, w1row, half, p_) awaiting wb+mo

        def emit_w1fin(gi):
            srt, half = fin.pop(gi)
            w1s = stg.tile([16, P], bf, tag="w1s", name="w1s")
            nc.vector.tensor_mul(out=w1s, in0=srt, in1=e1q[half])
            wqd = drp.tile([2048], bf, name=f"wqd{gi}")
            nc.sync.dma_start(
                out=wqd.rearrange("(r p) -> r p", r=16), in_=w1s
            )
            w1row = stg.tile([1, 2048], bf, tag="w1row", name="w1row")
            nc.sync.dma_start(
                out=w1row, in_=wqd.rearrange("(a x) -> a x", a=1)
            )
            usb, half_, p_ = pend[gi]
            pend[gi] = (usb, w1row, half_, p_)

        DR = mybir.MatmulPerfMode.DoubleRow

        def emit_av(p_, u_ps, expair, j):
            # one DoubleRow matmul per (head, qc) covers key-blocks 2j, 2j+1
            for h2 in range(2):
                for qc in range(2):
                    nc.tensor.matmul(
                        u_ps[h2 * 2 + qc],
                        vt8[j][:, :, p_ * 2 + h2, 0:65],
                        expair[h2][:, :, qc * 512:(qc + 1) * 512],
                        start=(j == 0),
                        stop=(j == 7),
                        perf_mode=DR,
                    )

        def emit_wbmo(gi):
            usb, w1row, half, p_ = pend.pop(gi)  # w1fin(gi) ran before this
            for h2 in range(2):
                for qc in range(2):
                    c = h2 * 2 + qc
                    c0 = half * 1024 + qc * 512
                    wb = scp.tile([64, 512], f32, tag="sc0", name="wb")
                    nc.tensor.matmul(
                        wb, ones1, w1row[0:1, c * 512:(c + 1) * 512],
                        start=True, stop=True,
                    )
                    nc.vector.tensor_mul(
                        out=attT[p_][h2 * 64:(h2 + 1) * 64, c0:c0 + 512],
                        in0=usb[c][0:64, :],
                        in1=wb,
                    )

        # Schraudolph fast-exp constants: exp(s*SCALE) ~= bitcast_f32(
        # int32(A*s + B)); B centers the relative error (rms ~1.8%), which
        # largely cancels in the softmax ratio. Applied to half of one head's
        # scores per iteration to split exp work between ScalarE and VectorE.
        S_A = SCALE * 12102203.1615614666
        S_B = 1064866805.0
        Mult = mybir.AluOpType.mult
        Add = mybir.AluOpType.add
        DCUT = 384  # sc1 cols [0:DCUT] exp'd on ScalarE, rest on VectorE

        for gi, (half, p_) in enumerate(quads):
            u_ps = [
                upp.tile([65, 512], f32, tag="u", name="u_ps")
                for _ in range(4)
            ]
            expair = None
            prevpair = None
            for kt in range(16):
                sc0 = scp.tile([P, 1024], f32, tag="sc0", name="sc0")
                sc1 = scp.tile([P, 1024], f32, tag="sc1", name="sc1")
                for n2 in range(2):
                    c0 = half * 1024 + n2 * 512
                    nc.tensor.matmul(
                        sc0[:, n2 * 512:(n2 + 1) * 512],
                        kT2[p_][0:64, kt * P:(kt + 1) * P],
                        qT2[p_][0:64, c0:c0 + 512],
                        start=True,
                        stop=True,
                    )
                    nc.tensor.matmul(
                        sc1[:, n2 * 512:(n2 + 1) * 512],
                        kT2[p_][64:128, kt * P:(kt + 1) * P],
                        qT2[p_][64:128, c0:c0 + 512],
                        start=True,
                        stop=True,
                    )
                if kt % 2 == 0:
                    if kt >= 2:
                        emit_av(p_, u_ps, prevpair, kt // 2 - 1)
                    prevpair = expair = (
                        exp_pool.tile([P, 2, 1024], f8, tag="ex", name="ext"),
                        exp_pool.tile([P, 2, 1024], f8, tag="ex", name="ext"),
                    )
                s = kt % 2
                nc.scalar.activation(
                    out=expair[0][:, s, :], in_=sc0, func=Exp, scale=SCALE
                )
                nc.scalar.activation(
                    out=expair[1][:, s, 0:DCUT], in_=sc1[:, 0:DCUT],
                    func=Exp, scale=SCALE,
                )
                t32 = stg.tile([P, 1024 - DCUT], mybir.dt.int32, tag="t32",
                               name="t32")
                nc.vector.tensor_scalar(
                    out=t32, in0=sc1[:, DCUT:1024],
                    scalar1=S_A, scalar2=S_B, op0=Mult, op1=Add,
                )
                nc.vector.tensor_copy(
                    out=expair[1][:, s, DCUT:1024], in_=t32[:, :].bitcast(f32)
                )
                if gi == 1:
                    if kt == 4:
                        emit_epi_a()
                    elif kt == 8:
                        emit_epi_b()
                    elif kt == 10:
                        emit_w1fin(0)
                if kt == 2 and (gi - 2) in pend:
                    emit_wbmo(gi - 2)
                if kt == 12 and gi >= 2 and (gi - 1) in pend:
                    emit_wbmo(gi - 1)
            emit_av(p_, u_ps, prevpair, 7)

            # quad drain: copy u out of PSUM; build w1 = e1/S off the PE.
            # All DMA hops use per-partition-contiguous runs (>=256B): the
            # sums go to DRAM as 4 rows, come back as [16,128] (partition =
            # (h2,qc,i), free = p), reciprocal+e1-mul run 16 lanes wide, and
            # the result stores straight back in (h2,qc,i,p)-linear order.
            usb = []
            sq = drp.tile([4, 512], bf, name=f"sq{gi}")
            for c in range(4):
                u_sb = usbp.tile([65, 512], bf, tag="usb", name="usb")
                nc.vector.tensor_copy(out=u_sb, in_=u_ps[c])
                nc.sync.dma_start(out=sq[c:c + 1, :], in_=u_sb[64:65, :])
                usb.append(u_sb)
            srt_b = stg.tile([16, P], bf, tag="srtb", name="srtb")
            nc.sync.dma_start(
                out=srt_b, in_=sq.rearrange("c (i p) -> (c i) p", p=P)
            )
            srt = stg.tile([16, P], f32, tag="srt", name="srt")
            nc.vector.reciprocal(out=srt, in_=srt_b)
            fin[gi] = (srt, half)
            pend[gi] = (usb, half, p_)
            if gi >= 1:
                emit_w1fin(gi)

        # ---- output projection (augmented with routing const rows) -------
        def emit_po(qt):
            po = scp.tile(
                [P, 1024], f32, tag=("sc0" if qt % 2 == 0 else "sc1"), name="po"
            )
            for n2 in range(2):
                ns = slice(n2 * 512, (n2 + 1) * 512)
                nc.tensor.matmul(
                    po[:, ns],
                    attT[0][:, qt * P:(qt + 1) * P],
                    wo_sb[:, 0, ns],
                    start=True,
                    stop=False,
                )
                nc.tensor.matmul(
                    po[:, ns],
                    attT[1][:, qt * P:(qt + 1) * P],
                    wo_sb[:, 1, ns],
                    start=False,
                    stop=False,
                )
                nc.tensor.matmul(
                    po[:, ns],
                    e_b[:, qt * P:(qt + 1) * P],
                    crow_sb[:, ns],
                    start=False,
                    stop=True,
                )
            ou = oup.tile([P, C], bf, tag="ou", name="ou")
            nc.scalar.activation(
                out=ou, in_=po, func=Copy, scale=s3rt[:, qt:qt + 1], bias=0.0
            )
            nc.sync.dma_start(out=outd[qt * P:(qt + 1) * P, :], in_=ou)

        for qt in range(8):
            emit_po(qt)
        emit_wbmo(3)
        for qt in range(8, 16):
            emit_po(qt)

    _split_excess_waits(nc, mybir)
    return nc


def _split_excess_waits(nc, mybir, keep=1):
    """Walrus in this build accepts at most 1 sync wait per instruction;
    move excess waits onto single-wait nops inserted just before, on the
    same engine (same-engine program order makes this equivalent)."""
    n_extra = 0
    for bb in nc.main_func.blocks:
        out = []
        for inst in bb.instructions:
            si = inst.sync_info
            if si is not None and si.on_wait and len(si.on_wait) > keep:
                waits = list(si.on_wait)
                si.on_wait = waits[:keep]
                for w in waits[keep:]:
                    n_extra += 1
                    nop = mybir.InstNoOp(
                        name=f"wsplit_{n_extra}_{inst.name}",
                        engine=inst.engine,
                        ins=[],
                        outs=[],
                        sync_info=mybir.SyncInfo(on_wait=[w], on_update=[]),
                    )
                    out.append(nop)
            out.append(inst)
        bb.instructions[:] = out
    return nc


def make_in_maps(inputs):
    import ml_dtypes

    bf16 = ml_dtypes.bfloat16
    f32 = np.float32
    x = np.asarray(inputs["hidden_states"], f32)
    Wq = np.asarray(inputs["Wq"], f32)
    bq = np.asarray(inputs["bq"], f32)
    Wk = np.asarray(inputs["Wk"], f32)
    Wv = np.asarray(inputs["Wv"], f32)
    bv = np.asarray(inputs["bv"], f32)
    Wo = np.asarray(inputs["Wo"], f32)
    bo = np.asarray(inputs["bo"], f32)
    Wr1 = np.asarray(inputs["Wr1"], f32)
    br1 = np.asarray(inputs["br1"], f32)
    Wr2 = np.asarray(inputs["Wr2"], f32)
    br2 = np.asarray(inputs["br2"], f32)

    xTb = [np.ascontiguousarray(x[b].T).astype(bf16) for b in range(B)]
    c3 = bv @ Wo + bo
    c2 = []
    for b in range(B):
        mean = x[b].mean(axis=0)
        c2.append((mean @ Wv + bv) @ Wo + bo)

    def cut(w, cs, ce):
        return np.ascontiguousarray(w[:, cs:ce]).astype(bf16)

    in_maps = []
    for c in range(8):
        b, h = divmod(c, 4)
        cs, ce = h * 256, h * 256 + 256
        crows_np = np.zeros((3, C), f32)
        crows_np[0] = bv[cs:ce] @ Wo[cs:ce, :] + (bo if h == 0 else 0.0)
        if h == 0:
            crows_np[1] = c2[b]
            crows_np[2] = c3
        in_maps.append({
            "xT": xTb[b],
            "wq": cut(Wq, cs, ce),
            "wk": cut(Wk, cs, ce),
            "wv": cut(Wv, cs, ce),
            "wr1": cut(Wr1, cs, ce),
            "wr2": np.ascontiguousarray(Wr2[cs:ce, :]).astype(bf16),
            "wo": np.ascontiguousarray(Wo[cs:ce, :]).astype(bf16),
            "crows": crows_np.astype(bf16),
            "bq2": np.ascontiguousarray(bq[cs:ce].reshape(2, P, 1)),
            "br1t": np.ascontiguousarray(br1[cs:ce].reshape(2, P, 1)),
            "br2z": np.ascontiguousarray(
                (br2 if h == 0 else np.zeros(3, f32)).reshape(3, 1)
            ),
        })
    return in_maps


_NC = None


def kernel(**inputs):
    global _NC
    _setup_env()
    from concourse.bass_utils import run_bass_kernel_spmd

    if _NC is None:
        _NC = build_nc()
    in_maps = make_in_maps(inputs)
    res = run_bass_kernel_spmd(_NC, in_maps, core_ids=list(range(8)))
    outs = [res.results[c]["out"].astype(np.float32) for c in range(8)]
    full = np.stack(
        [outs[0] + outs[1] + outs[2] + outs[3],
         outs[4] + outs[5] + outs[6] + outs[7]],
        axis=0,
    )
    return full


# revision 45
# speedup vs baseline: 4.0417x; 1.0208x over previous
"""Multi-level-KV attention (MKA) kernel for 8 TRN2 NeuronCores.

Math shortcut: memory levels L2 (mean-pooled, all keys identical) and L3
(zeros) have exactly uniform attention, so their contributions collapse to
per-batch constant vectors folded into 3 augmented rows of the output
projection. Only L1 needs real attention.

Sharding: core c -> batch b=c//4, head-quad h=c%4 (4 of 16 heads; column
slice 256h:256h+256 of the q/k/v projections, row slice of Wo). The routing
MLP is column-sharded with a tiny [3,2048] logits AllReduce per batch group.
Each core returns a partial [T,C] output; the host sums each group of 4.

Schedule notes (v2): inputs arrive in a handful of batched DMAs so the PE
starts within a few us; the attention kt-loop is software-pipelined (AV
matmuls consume the previous iteration's exp tiles) so the PE stream has no
intra-loop waits; all per-quad softmax normalization runs off the PE via a
DRAM-bounce transpose + one [128,16] reciprocal; the normalizer broadcast
(wb) and attT writeback (mo) for quad g are emitted mid-quad g+1 so their
DMA chain never stalls the PE. Output partials are bf16, summed on host.
"""
import sys

import numpy as np

_REPO = "/opt/trn_rl_repo"

B, T, C, H = 2, 2048, 1024, 16
D = C // H
P = 128
SCALE = D ** -0.5
NHL = 4  # heads per core


def _setup_env():
    if _REPO not in sys.path:
        sys.path.insert(0, _REPO)
    import concourse.tile as tile
    from concourse import mybir
    from concourse.vector_clock import ScopedClock

    if getattr(tile.TileContext, "_drain_patched", False):
        return

    # This walrus build rejects CTRL instructions (Drain) carrying more than
    # one sync wait; move the end-of-kernel drain's waits onto 1-wait nops.
    def _drain_and_barrier_split(self, tick_clock, wait_clock):
        carrier = self.nc.sync.nop(nofuse=True, hint="drain_wait_carrier")
        wait_clock.add_sem_waits(
            carrier.ins, ScopedClock({None: tick_clock.global_clock})
        )
        si = carrier.ins.sync_info
        waits = list(si.on_wait) if si is not None and si.on_wait else []
        if si is not None:
            si.on_wait = waits[:1]
        for w in waits[1:]:
            nop = self.nc.sync.nop(nofuse=True, hint="drain_wait_carrier")
            nop.ins.sync_info = mybir.SyncInfo(on_wait=[w], on_update=[])
        self.nc.sync.drain()
        self.nc.all_engine_barrier()
        assert self.sems is not None
        popped = self.nc._tile_sem_poison_stack.pop()
        assert popped is self._sem_poison
        self.nc.clear_and_free_semaphores(list(self.sems.allocated().values()))
        self.nc.all_engine_barrier()

    tile.TileContext._drain_and_barrier = _drain_and_barrier_split
    tile.TileContext._drain_patched = True


def build_nc():
    _setup_env()
    from contextlib import ExitStack

    import concourse.bass as bass
    import concourse.tile as tile
    from concourse import mybir

    f32 = mybir.dt.float32
    bf = mybir.dt.bfloat16
    Exp = mybir.ActivationFunctionType.Exp
    Tanh = mybir.ActivationFunctionType.Tanh
    Copy = mybir.ActivationFunctionType.Copy

    f8d = mybir.dt.float8e4
    nc = bass.Bass()
    xT = nc.dram_tensor("xT", [C, T], f8d, kind="ExternalInput")
    wq = nc.dram_tensor("wq", [C, 256], f8d, kind="ExternalInput")
    wk = nc.dram_tensor("wk", [C, 256], f8d, kind="ExternalInput")
    wv = nc.dram_tensor("wv", [C, 256], f8d, kind="ExternalInput")
    wr1 = nc.dram_tensor("wr1", [C, 256], f8d, kind="ExternalInput")
    wr2 = nc.dram_tensor("wr2", [256, 3], bf, kind="ExternalInput")
    wo = nc.dram_tensor("wo", [256, C], f8d, kind="ExternalInput")
    crows = nc.dram_tensor("crows", [3, C], bf, kind="ExternalInput")
    bq2 = nc.dram_tensor("bq2", [2, P, 1], f32, kind="ExternalInput")
    br1t = nc.dram_tensor("br1t", [2, P, 1], f32, kind="ExternalInput")
    br2z = nc.dram_tensor("br2z", [3, 1], f32, kind="ExternalInput")
    outd = nc.dram_tensor("out", [T, C], bf, kind="ExternalOutput")

    with tile.TileContext(nc) as tc, ExitStack() as ctx:
        pp = ctx.enter_context(tc.tile_pool(name="persist", bufs=1))
        exp_pool = ctx.enter_context(tc.tile_pool(name="expool", bufs=6))
        scp = ctx.enter_context(tc.tile_pool(name="scp", bufs=1, space="PSUM"))
        upp = ctx.enter_context(tc.tile_pool(name="upp", bufs=4, space="PSUM"))
        stg = ctx.enter_context(tc.tile_pool(name="stg", bufs=5))
        usbp = ctx.enter_context(tc.tile_pool(name="usbp", bufs=9))
        oup = ctx.enter_context(tc.tile_pool(name="oup", bufs=3))
        drp = ctx.enter_context(tc.tile_pool(name="drm", bufs=1, space="DRAM"))

        # ---- batched persistent loads -------------------------------------
        # xt4[ch][p, k, t] = xT[k*128+p, ch*512+t]  (token-chunk major)
        # small tensors first: the tanh bias must not queue behind megabyte
        # loads in the Sync FIFO (it gated the whole routing chain once)
        bq_sb = pp.tile([P, 2], f32, name="bq_sb")
        nc.sync.dma_start(out=bq_sb, in_=bq2.rearrange("a p o -> p (a o)"))
        br1_sb = pp.tile([P, 2], f32, name="br1_sb")
        nc.sync.dma_start(out=br1_sb, in_=br1t.rearrange("a p o -> p (a o)"))
        br2_sb = pp.tile([3, 1], f32, name="br2_sb")
        nc.sync.dma_start(out=br2_sb, in_=br2z[:, :])
        wr2_sb = pp.tile([P, 2, 3], bf, name="wr2_sb")
        nc.sync.dma_start(out=wr2_sb, in_=wr2.rearrange("(a p) b -> p a b", p=P))
        crow_sb = pp.tile([3, C], bf, name="crow_sb")
        nc.sync.dma_start(out=crow_sb, in_=crows[:, :])

        # fp8 operands in DoubleRow pair layout: k-tile pairs (2k2, 2k2+1)
        # land as the `s` dim so one DR matmul contracts 256 rows per pass
        f8 = mybir.dt.float8e4
        xr = xT.rearrange("(k2 s p) t -> p s k2 t", s=2, p=P)

        def load_pairs(tile_, w_, cols):
            wr_ = w_.rearrange("(k2 s p) c -> p s k2 c", s=2, p=P)
            for s in range(2):
                nc.sync.dma_start(out=tile_[:, s, :, :], in_=wr_[:, s, :, :])

        xt4 = []
        for ch in range(4):
            t_ = pp.tile([P, 2, 4, 512], f8, name=f"xt4_{ch}")
            xt4.append(t_)
        for s in range(2):
            nc.sync.dma_start(
                out=xt4[0][:, s, :, :], in_=xr[:, s, :, 0:512]
            )
        wr1_sb = pp.tile([P, 2, 4, 256], f8, name="wr1sb")
        load_pairs(wr1_sb, wr1, 256)
        for ch in range(1, 4):
            for s in range(2):
                nc.sync.dma_start(
                    out=xt4[ch][:, s, :, :],
                    in_=xr[:, s, :, ch * 512:(ch + 1) * 512],
                )
        wq_sb = pp.tile([P, 2, 4, 256], f8, name="wqsb")
        load_pairs(wq_sb, wq, 256)
        wk_sb = pp.tile([P, 2, 4, 256], f8, name="wksb")
        load_pairs(wk_sb, wk, 256)
        wv_sb = pp.tile([P, 2, 4, 256], f8, name="wvsb")
        load_pairs(wv_sb, wv, 256)
        wo_sb = pp.tile([P, 2, C], f8, name="wosb")
        nc.sync.dma_start(
            out=wo_sb, in_=wo.rearrange("(s p) c -> p s c", s=2, p=P)
        )

        DR = mybir.MatmulPerfMode.DoubleRow
        qT2 = [pp.tile([P, T], bf, name=f"qT{p}") for p in range(2)]
        kT2 = [pp.tile([P, T], bf, name=f"kT{p}") for p in range(2)]
        # attT in fp8 with the two 128-dim blocks as the DR pair dim
        attT8 = pp.tile([P, 2, T], f8, name="attT8")
        # v in fp8, key-blocks paired for DoubleRow AV; m padded to 68 so the
        # pair stride (4*68 bytes) is 16-aligned as checkMatmultPerfMode wants
        vt8 = [pp.tile([P, 2, NHL, 68], f8, name=f"vt8_{j}") for j in range(8)]
        gt = [pp.tile([P, T], bf, name=f"gt{m}") for m in range(2)]
        e_f = pp.tile([3, T], f32, name="e_f")
        e_b = pp.tile([3, T], bf, name="e_b")
        lg_sb = pp.tile([3, T], f32, name="lg_sb")
        lgr_sb = pp.tile([3, T], f32, name="lgr_sb")
        e3t = pp.tile([P, 48], f32, name="e3t")
        s3rt = pp.tile([P, 16], f32, name="s3rt")
        e1q = [pp.tile([16, P], f32, name=f"e1q{h}") for h in range(2)]
        ones1 = pp.tile([1, 64], bf, name="ones1")
        nc.vector.memset(ones1, 1.0)

        # ---- routing: gT = tanh(Wr1_blk^T xT), partial logits, AllReduce --
        for m in range(2):
            for ch in range(4):
                g_ps = upp.tile([P, 512], f32, tag="u", name="g_ps")
                for k2 in range(4):
                    nc.tensor.matmul(
                        g_ps,
                        wr1_sb[:, :, k2, m * P:(m + 1) * P],
                        xt4[ch][:, :, k2, :],
                        start=(k2 == 0),
                        stop=(k2 == 3),
                        perf_mode=DR,
                    )
                nc.scalar.activation(
                    out=gt[m][:, ch * 512:(ch + 1) * 512],
                    in_=g_ps,
                    func=Tanh,
                    bias=br1_sb[:, m:m + 1],
                    scale=1.0,
                )
        # ---- projections: qT/kT (heads stacked in pairs), v_aug.
        # The routing logits + AllReduce are emitted after two q/k chunk
        # pairs so the PE queue never head-blocks on the gt tanh latency. ---
        def emit_qk(p_, ch):
            q_ps = upp.tile([P, 512], f32, tag="u", name="q_ps")
            for k2 in range(4):
                nc.tensor.matmul(
                    q_ps,
                    wq_sb[:, :, k2, p_ * P:(p_ + 1) * P],
                    xt4[ch][:, :, k2, :],
                    start=(k2 == 0),
                    stop=(k2 == 3),
                    perf_mode=DR,
                )
            nc.vector.tensor_scalar_add(
                out=qT2[p_][:, ch * 512:(ch + 1) * 512],
                in0=q_ps,
                scalar1=bq_sb[:, p_:p_ + 1],
            )
            k_ps = upp.tile([P, 512], f32, tag="u", name="k_ps")
            for k2 in range(4):
                nc.tensor.matmul(
                    k_ps,
                    wk_sb[:, :, k2, p_ * P:(p_ + 1) * P],
                    xt4[ch][:, :, k2, :],
                    start=(k2 == 0),
                    stop=(k2 == 3),
                    perf_mode=DR,
                )
            nc.vector.tensor_copy(
                out=kT2[p_][:, ch * 512:(ch + 1) * 512], in_=k_ps
            )

        emit_qk(0, 0)
        for ch in range(4):
            lg_ps = upp.tile([3, 512], f32, tag="u", name="lg_ps")
            for a in range(2):
                nc.tensor.matmul(
                    lg_ps,
                    wr2_sb[:, a, :],
                    gt[a][:, ch * 512:(ch + 1) * 512],
                    start=(a == 0),
                    stop=(a == 1),
                )
            nc.vector.tensor_scalar_add(
                out=lg_sb[:, ch * 512:(ch + 1) * 512],
                in0=lg_ps,
                scalar1=br2_sb,
            )
        lg_in = drp.tile([3, T], f32, name="lg_in")
        lg_out = drp.tile([3, T], f32, name="lg_out")
        nc.sync.dma_start(out=lg_in, in_=lg_sb)
        nc.gpsimd.collective_compute(
            "AllReduce",
            mybir.AluOpType.add,
            replica_groups=[[0, 1, 2, 3], [4, 5, 6, 7]],
            ins=[lg_in.opt()],
            outs=[lg_out.opt()],
        )
        nc.sync.dma_start(out=lgr_sb, in_=lg_out)
        for ch in range(1, 4):
            emit_qk(0, ch)
        for ch in range(4):
            emit_qk(1, ch)
        for j in range(16):
            v_ps = upp.tile([P, 256], f32, tag="u", name="v_ps")
            for k2 in range(4):
                nc.tensor.matmul(
                    v_ps,
                    xt4[j // 4][:, :, k2, (j % 4) * P:(j % 4) * P + P],
                    wv_sb[:, :, k2, :],
                    start=(k2 == 0),
                    stop=(k2 == 3),
                    perf_mode=DR,
                )
            nc.vector.tensor_copy(
                out=vt8[j // 2][:, j % 2, :, 0:64],
                in_=v_ps.rearrange("p (h d) -> p h d", h=NHL),
            )
            nc.vector.memset(vt8[j // 2][:, j % 2, :, 64:65], 1.0)

        # routing epilogue pieces — emitted staged inside quad g1 so the
        # AllReduce latency never head-blocks the Scalar/Vector FIFOs.
        e3d = drp.tile([3, T], f32, name="e3d")

        def emit_epi_a():
            nc.scalar.activation(out=e_f, in_=lgr_sb, func=Exp, scale=1.0)
            nc.vector.tensor_copy(out=e_b, in_=e_f)
            nc.sync.dma_start(out=e3d, in_=e_f)
            nc.sync.dma_start(
                out=e3t, in_=e3d.rearrange("a (i p) -> p (a i)", p=P)
            )

        def emit_epi_b():
            nc.vector.tensor_add(out=s3rt, in0=e3t[:, 0:16], in1=e3t[:, 16:32])
            nc.vector.tensor_add(out=s3rt, in0=s3rt, in1=e3t[:, 32:48])
            nc.vector.reciprocal(out=s3rt, in_=s3rt)
            # e1 in quad-drain form: partition (h2,qc,i), free p (see drain)
            for h in range(2):
                src = e3d[0:1, h * 1024:(h + 1) * 1024]
                for rep in range(2):
                    nc.sync.dma_start(
                        out=e1q[h][rep * 8:rep * 8 + 8, :],
                        in_=src.rearrange(
                            "a (q i p) -> (a q i) p", q=2, i=4, p=P
                        ),
                    )

        # ---- attention: half-major quads, software-pipelined kt loop ------
        quads = [(h, p_) for h in range(2) for p_ in range(2)]
        fin = {}   # quad idx -> (srt, half) awaiting the e1-mul + return DMAs
        pend = {}  # quad idx -> (usb list, w1row, half, p_) awaiting wb+mo

        def emit_w1fin(gi):
            srt, half = fin.pop(gi)
            w1s = stg.tile([16, P], bf, tag="w1s", name="w1s")
            nc.vector.tensor_mul(out=w1s, in0=srt, in1=e1q[half])
            wqd = drp.tile([2048], bf, name=f"wqd{gi}")
            nc.sync.dma_start(
                out=wqd.rearrange("(r p) -> r p", r=16), in_=w1s
            )
            w1row = stg.tile([1, 2048], bf, tag="w1row", name="w1row")
            nc.sync.dma_start(
                out=w1row, in_=wqd.rearrange("(a x) -> a x", a=1)
            )
            usb, half_, p_ = pend[gi]
            pend[gi] = (usb, w1row, half_, p_)

        DR = mybir.MatmulPerfMode.DoubleRow

        def emit_av(p_, u_ps, expair, j):
            # one DoubleRow matmul per (head, qc) covers key-blocks 2j, 2j+1
            for h2 in range(2):
                for qc in range(2):
                    nc.tensor.matmul(
                        u_ps[h2 * 2 + qc],
                        vt8[j][:, :, p_ * 2 + h2, 0:65],
                        expair[h2][:, :, qc * 512:(qc + 1) * 512],
                        start=(j == 0),
                        stop=(j == 7),
                        perf_mode=DR,
                    )

        def emit_wbmo(gi):
            usb, w1row, half, p_ = pend.pop(gi)  # w1fin(gi) ran before this
            for h2 in range(2):
                for qc in range(2):
                    c = h2 * 2 + qc
                    c0 = half * 1024 + qc * 512
                    wb = scp.tile([64, 512], f32, tag="sc0", name="wb")
                    nc.tensor.matmul(
                        wb, ones1, w1row[0:1, c * 512:(c + 1) * 512],
                        start=True, stop=True,
                    )
                    nc.vector.tensor_mul(
                        out=attT8[h2 * 64:(h2 + 1) * 64, p_, c0:c0 + 512],
                        in0=usb[c][0:64, :],
                        in1=wb,
                    )

        # Schraudolph fast-exp constants: exp(s*SCALE) ~= bitcast_f32(
        # int32(A*s + B)); B centers the relative error (rms ~1.8%), which
        # largely cancels in the softmax ratio. Applied to half of one head's
        # scores per iteration to split exp work between ScalarE and VectorE.
        # fp8 variant of the bit trick: e4m3 bits = round(8*log2(e^x)) + bias,
        # computed as one tensor_scalar into an int8-bitcast view of the exp
        # tile. The linear-mantissa error is below e4m3's own quantization.
        I_A = SCALE * 8 / 0.6931471805599453
        I_B = 56.0 - 486411.0 / 2 ** 20
        Mult = mybir.AluOpType.mult
        Add = mybir.AluOpType.add
        DCUT = 192  # sc1 cols [0:DCUT] exp'd on ScalarE, rest on VectorE

        for gi, (half, p_) in enumerate(quads):
            u_ps = [
                upp.tile([65, 512], f32, tag="u", name="u_ps")
                for _ in range(4)
            ]
            expair = None
            prevpair = None
            for kt in range(16):
                sc0 = scp.tile([P, 1024], f32, tag="sc0", name="sc0")
                sc1 = scp.tile([P, 1024], f32, tag="sc1", name="sc1")
                for n2 in range(2):
                    c0 = half * 1024 + n2 * 512
                    nc.tensor.matmul(
                        sc0[:, n2 * 512:(n2 + 1) * 512],
                        kT2[p_][0:64, kt * P:(kt + 1) * P],
                        qT2[p_][0:64, c0:c0 + 512],
                        start=True,
                        stop=True,
                    )
                    nc.tensor.matmul(
                        sc1[:, n2 * 512:(n2 + 1) * 512],
                        kT2[p_][64:128, kt * P:(kt + 1) * P],
                        qT2[p_][64:128, c0:c0 + 512],
                        start=True,
                        stop=True,
                    )
                if kt % 2 == 0:
                    if kt >= 2:
                        emit_av(p_, u_ps, prevpair, kt // 2 - 1)
                    prevpair = expair = (
                        exp_pool.tile([P, 2, 1024], f8, tag="ex", name="ext"),
                        exp_pool.tile([P, 2, 1024], f8, tag="ex", name="ext"),
                    )
                s = kt % 2
                nc.scalar.activation(
                    out=expair[0][:, s, :], in_=sc0, func=Exp, scale=SCALE
                )
                nc.scalar.activation(
                    out=expair[1][:, s, 0:DCUT], in_=sc1[:, 0:DCUT],
                    func=Exp, scale=SCALE,
                )
                nc.vector.tensor_scalar(
                    out=expair[1][:, s, DCUT:1024].bitcast(mybir.dt.uint8),
                    in0=sc1[:, DCUT:1024],
                    scalar1=I_A, scalar2=I_B, op0=Mult, op1=Add,
                )
                if gi == 1:
                    if kt == 4:
                        emit_epi_a()
                    elif kt == 8:
                        emit_epi_b()
                    elif kt == 10:
                        emit_w1fin(0)
                if kt == 2 and (gi - 2) in pend:
                    emit_wbmo(gi - 2)
                if kt == 12 and gi >= 2 and (gi - 1) in pend:
                    emit_wbmo(gi - 1)
            emit_av(p_, u_ps, prevpair, 7)

            # quad drain: copy u out of PSUM; build w1 = e1/S off the PE.
            # All DMA hops use per-partition-contiguous runs (>=256B): the
            # sums go to DRAM as 4 rows, come back as [16,128] (partition =
            # (h2,qc,i), free = p), reciprocal+e1-mul run 16 lanes wide, and
            # the result stores straight back in (h2,qc,i,p)-linear order.
            usb = []
            sq = drp.tile([4, 512], bf, name=f"sq{gi}")
            for c in range(4):
                u_sb = usbp.tile([65, 512], bf, tag="usb", name="usb")
                nc.vector.tensor_copy(out=u_sb, in_=u_ps[c])
                nc.sync.dma_start(out=sq[c:c + 1, :], in_=u_sb[64:65, :])
                usb.append(u_sb)
            srt_b = stg.tile([16, P], bf, tag="srtb", name="srtb")
            nc.sync.dma_start(
                out=srt_b, in_=sq.rearrange("c (i p) -> (c i) p", p=P)
            )
            srt = stg.tile([16, P], f32, tag="srt", name="srt")
            nc.vector.reciprocal(out=srt, in_=srt_b)
            fin[gi] = (srt, half)
            pend[gi] = (usb, half, p_)
            if gi >= 1:
                emit_w1fin(gi)

        # ---- output projection (augmented with routing const rows) -------
        def emit_po(qt):
            po = scp.tile(
                [P, 1024], f32, tag=("sc0" if qt % 2 == 0 else "sc1"), name="po"
            )
            for n2 in range(2):
                ns = slice(n2 * 512, (n2 + 1) * 512)
                nc.tensor.matmul(
                    po[:, ns],
                    attT8[:, :, qt * P:(qt + 1) * P],
                    wo_sb[:, :, ns],
                    start=True,
                    stop=False,
                    perf_mode=DR,
                )
                nc.tensor.matmul(
                    po[:, ns],
                    e_b[:, qt * P:(qt + 1) * P],
                    crow_sb[:, ns],
                    start=False,
                    stop=True,
                )
            ou = oup.tile([P, C], bf, tag="ou", name="ou")
            nc.scalar.activation(
                out=ou, in_=po, func=Copy, scale=s3rt[:, qt:qt + 1], bias=0.0
            )
            nc.sync.dma_start(out=outd[qt * P:(qt + 1) * P, :], in_=ou)

        for qt in range(8):
            emit_po(qt)
        emit_wbmo(3)
        for qt in range(8, 16):
            emit_po(qt)

    _split_excess_waits(nc, mybir)
    return nc


def _split_excess_waits(nc, mybir, keep=1):
    """Walrus in this build accepts at most 1 sync wait per instruction;
    move excess waits onto single-wait nops inserted just before, on the
    same engine (same-engine program order makes this equivalent)."""
    n_extra = 0
    for bb in nc.main_func.blocks:
        out = []
        for inst in bb.instructions:
            si = inst.sync_info
            if si is not None and si.on_wait and len(si.on_wait) > keep:
                waits = list(si.on_wait)
                si.on_wait = waits[:keep]
                for w in waits[keep:]:
                    n_extra += 1
                    nop = mybir.InstNoOp(
                        name=f"wsplit_{n_extra}_{inst.name}",
                        engine=inst.engine,
                        ins=[],
                        outs=[],
                        sync_info=mybir.SyncInfo(on_wait=[w], on_update=[]),
                    )
                    out.append(nop)
            out.append(inst)
        bb.instructions[:] = out
    return nc


def make_in_maps(inputs):
    import ml_dtypes

    bf16 = ml_dtypes.bfloat16
    f8 = ml_dtypes.float8_e4m3
    f32 = np.float32
    x = np.asarray(inputs["hidden_states"], f32)
    Wq = np.asarray(inputs["Wq"], f32)
    bq = np.asarray(inputs["bq"], f32)
    Wk = np.asarray(inputs["Wk"], f32)
    Wv = np.asarray(inputs["Wv"], f32)
    bv = np.asarray(inputs["bv"], f32)
    Wo = np.asarray(inputs["Wo"], f32)
    bo = np.asarray(inputs["bo"], f32)
    Wr1 = np.asarray(inputs["Wr1"], f32)
    br1 = np.asarray(inputs["br1"], f32)
    Wr2 = np.asarray(inputs["Wr2"], f32)
    br2 = np.asarray(inputs["br2"], f32)

    xTb = [np.ascontiguousarray(x[b].T).astype(f8) for b in range(B)]
    c3 = bv @ Wo + bo
    c2 = []
    for b in range(B):
        mean = x[b].mean(axis=0)
        c2.append((mean @ Wv + bv) @ Wo + bo)

    def cut(w, cs, ce):
        return np.ascontiguousarray(w[:, cs:ce]).astype(f8)

    in_maps = []
    for c in range(8):
        b, h = divmod(c, 4)
        cs, ce = h * 256, h * 256 + 256
        crows_np = np.zeros((3, C), f32)
        crows_np[0] = bv[cs:ce] @ Wo[cs:ce, :] + (bo if h == 0 else 0.0)
        if h == 0:
            crows_np[1] = c2[b]
            crows_np[2] = c3
        in_maps.append({
            "xT": xTb[b],
            "wq": cut(Wq, cs, ce),
            "wk": cut(Wk, cs, ce),
            "wv": cut(Wv, cs, ce),
            "wr1": cut(Wr1, cs, ce),
            "wr2": np.ascontiguousarray(Wr2[cs:ce, :]).astype(bf16),
            "wo": np.ascontiguousarray(Wo[cs:ce, :]).astype(f8),
            "crows": crows_np.astype(bf16),
            "bq2": np.ascontiguousarray(bq[cs:ce].reshape(2, P, 1)),
            "br1t": np.ascontiguousarray(br1[cs:ce].reshape(2, P, 1)),
            "br2z": np.ascontiguousarray(
                (br2 if h == 0 else np.zeros(3, f32)).reshape(3, 1)
            ),
        })
    return in_maps


_NC = None


def kernel(**inputs):
    global _NC
    _setup_env()
    from concourse.bass_utils import run_bass_kernel_spmd

    if _NC is None:
        _NC = build_nc()
    in_maps = make_in_maps(inputs)
    res = run_bass_kernel_spmd(_NC, in_maps, core_ids=list(range(8)))
    outs = [res.results[c]["out"].astype(np.float32) for c in range(8)]
    full = np.stack(
        [outs[0] + outs[1] + outs[2] + outs[3],
         outs[4] + outs[5] + outs[6] + outs[7]],
        axis=0,
    )
    return full
